# revision 46
# baseline (speedup 1.0000x reference)
"""Trainium2 Bass kernel for nn_L2MLoRA (fused linear + routed LoRA).

Math (per batch element b, with e = idx[b,0]):
    y[b] = x[b] @ W.T + bias + SCALE * (x[b] @ A_pool[e]) @ B_pool[e]

Strategy: data-parallel over batch B=8 -> one batch element per NeuronCore.
Since each core handles exactly one expert, the rank-8 LoRA update is folded
into the base weight on the host:

    W_eff_c = W + SCALE * (A_pool[e_c] @ B_pool[e_c]).T        # [DIM, DIM]
    y[c]    = x[c] @ W_eff_c.T + bias

so the device program is a single dense linear layer. Everything is computed
in the transposed domain (yT = W_eff @ xT) so all matmul operands already
have the contraction dim on partitions and no on-device transposes are
needed. Operands are bf16 (same 1 cycle/row PE rate as fp32r, half the DMA
traffic); PSUM accumulates in fp32. Bias is applied by ScalarE during the
PSUM->SBUF copy, which also narrows to bf16 for the store (host widens back
to fp32).

Single-shot startup: the PE is pre-warmed with dummy matmuls on a zeroed
scratch tile (stalls reset the PE DVFS p-state: a cold matmul row costs
~3.7x a ramped one), while x chunk 0 (in quarters) and the k-major weight
chunks stream in an explicit just-in-time order on one HWDGE ring, so
arrival order is deterministic. Chunk 0 is then computed k-outer/o-inner
across all 8 PSUM banks, so each k step needs only the (x, W) k-pieces
already landed: PE consumption (~1.7us/k) outpaces DMA arrival (~1us/k)
and the PE never starves again. The final output group runs in 128-token
slivers so the end-of-kernel store + completion chain starts early.
"""

import numpy as np
import ml_dtypes

import concourse.bass as bass
import concourse.tile as tile
from concourse import bacc, mybir
from concourse.bass_utils import run_bass_kernel_spmd

B, N, DIM, POOL, RANK = 8, 2048, 1024, 64, 8
SCALE = 2.0
NCORES = 8
P = 128          # partitions / k-tile height / o-chunk width
TW = 512         # token-chunk width (max f32 moving free dim = PSUM bank)
KT = DIM // P    # 8 k-tiles over the contraction dim
OT = DIM // P    # 8 output chunks
TT = N // TW     # 4 token chunks
F32 = mybir.dt.float32
BF16 = mybir.dt.bfloat16
BF16_NP = ml_dtypes.bfloat16


def build_program(n_iter: int = 1, probe: str = "full"):
    """Build the single-core Tile program (same program runs SPMD on 8 cores).

    n_iter > 1 wraps the body in a For_i loop for benchmarking.
    probe: "full" | "nodma" (x resident, no stores) | "dmaonly" (no matmuls).
    """
    nc = bacc.Bacc("TRN2", target_bir_lowering=False, debug=False,
                   num_devices=NCORES)

    # xt[t, p, k*TW + j] = x[token t*TW+j, dim k*P+p] -> one 1MB DMA per t
    x_d = nc.dram_tensor("xt", [TT, P, KT * TW], BF16, kind="ExternalInput")
    # k-major weights: wt[k, p, o*P + c] = W_eff[o*P+c, k*P+p]
    w_d = nc.dram_tensor("wt", [KT, P, OT * P], BF16, kind="ExternalInput")
    bias_d = nc.dram_tensor("bias", [P, OT], F32, kind="ExternalInput")
    # y[t, o, p, j] = y[token t*TW+j, out o*P+p] -> contiguous 128KB per (t,o)
    y_d = nc.dram_tensor("y", [TT, OT, P, TW], BF16, kind="ExternalOutput")

    single = n_iter == 1 and probe != "nodma"

    with tile.TileContext(nc) as tc:
        with (
            tc.tile_pool(name="cpool", bufs=1) as cpool,
            tc.tile_pool(name="xpool",
                         bufs=(TT + 1 if probe == "nodma" else
                               3 if single else 4)) as xpool,
            tc.tile_pool(name="qpool", bufs=4) as qpool,
            tc.tile_pool(name="opool", bufs=4) as opool,
            tc.tile_pool(name="psy", bufs=8, space="PSUM") as psy_pool,
        ):
            # Loop path: x loads + y stores ride the SP HWDGE ring, the
            # weight/bias prologue rides the Activation ring. Single-shot
            # ordering is handled explicitly below.
            def load_xt(t):
                xx = xpool.tile([P, KT * TW], BF16, tag="xx")
                nc.sync.dma_start(xx[:], x_d.ap()[t])
                return xx

            def psum_tile():
                # single allocation site: one 2KB/partition slot x 8 bufs
                # fills the 8 PSUM banks exactly
                return psy_pool.tile([P, TW], F32, name="ps", tag="ps")

            bias_sb = cpool.tile([P, OT], F32, tag="bias")
            w_sb = [cpool.tile([P, OT * P], BF16, tag=f"w{k}", name=f"w{k}")
                    for k in range(KT)]
            w0_halves = None
            first_eighths = None
            if single:
                # bias rides the Pool SWDGE path: off the load ring
                nc.gpsimd.dma_start(bias_sb[:], bias_d.ap()[:])
                # PE warmup on a zeroed scratch tile; results go to PSUM
                # banks that every real accumulation group later resets.
                scratch = cpool.tile([P, P], BF16, tag="scratch")
                nc.vector.memset(scratch[:], 0)
                for _ in range(20):
                    ps_d = psum_tile()
                    nc.tensor.matmul(ps_d[:, :P], scratch[:], scratch[:],
                                     start=True, stop=True)
                # Single-shot: ALL loads ride the SP ring in an explicit
                # just-in-time order (HWDGE FIFO = program order): x0
                # quarters interleaved with the first weight k-chunks, the
                # remaining weights, then the later x chunks. One ring makes
                # arrival order deterministic; two rings race for the shared
                # SDMA engines and bulk x transfers crowd out weight chunks.
                # wk0 alone rides the Act ring, split in halves, so the PE
                # start gate is q0 + a 128KB half-chunk; on HW its transfer
                # overlaps q0's on the parallel SDMA engines.
                HWC = OT * P // 2
                w0a = cpool.tile([P, HWC], BF16, tag="w0a")
                w0b = cpool.tile([P, HWC], BF16, tag="w0b")
                w0_halves = (w0a, w0b)
                nc.scalar.dma_start(w0a[:], w_d.ap()[0][:, :HWC])
                nc.scalar.dma_start(w0b[:], w_d.ap()[0][:, HWC:])
                first_eighths = []
                for q in range(4):
                    xe = qpool.tile([P, 2 * TW], BF16, tag="xq", name="xq")
                    nc.sync.dma_start(
                        xe[:], x_d.ap()[0][:, q * 2 * TW:(q + 1) * 2 * TW])
                    first_eighths.append((xe, 2 * q, 2))
                    if q > 0:
                        nc.sync.dma_start(w_sb[q][:], w_d.ap()[q])
                for k in range(4, KT):
                    nc.sync.dma_start(w_sb[k][:], w_d.ap()[k])
            else:
                nc.scalar.dma_start(bias_sb[:], bias_d.ap()[:])
                for k in range(KT):
                    nc.scalar.dma_start(w_sb[k][:], w_d.ap()[k])

            if probe == "nodma":
                resident = [load_xt(t) for t in range(TT)]

            def w_slice(o, k):
                if k == 0 and w0_halves is not None:
                    return w0_halves[o // 4][:, (o % 4) * P:(o % 4 + 1) * P]
                return w_sb[k][:, o * P:(o + 1) * P]

            def rhs_slice(xt, k):
                if isinstance(xt, list):  # chunk held as k-ordered pieces
                    for tile_, k0, n_k in xt:
                        if k0 <= k < k0 + n_k:
                            return tile_[:, (k - k0) * TW:(k - k0 + 1) * TW]
                    raise AssertionError(f"no piece covers k={k}")
                return xt[:, k * TW:(k + 1) * TW]

            def act_store(t, o, ps):
                ob = opool.tile([P, TW], BF16, tag="ob")
                nc.scalar.activation(
                    ob[:], ps[:],
                    mybir.ActivationFunctionType.Identity,
                    bias=bias_sb[:, o:o + 1], scale=1.0,
                )
                if probe != "nodma":
                    nc.sync.dma_start(y_d.ap()[t, o], ob[:])

            def compute_chunk(t, xt):
                # steady state: o-outer, k-inner (one PSUM bank at a time)
                for o in range(OT):
                    if single and t == TT - 1 and o == OT - 1:
                        # last group in 128-token slivers: the final matmul
                        # before the drain chain is tiny, so act + store +
                        # DMA-completion start ~1.5us earlier
                        ob = opool.tile([P, TW], BF16, tag="ob")
                        for s in range(4):
                            sl = slice(s * P, (s + 1) * P)
                            ps = psum_tile()
                            for k in range(KT):
                                nc.tensor.matmul(
                                    ps[:, :P], w_slice(o, k),
                                    rhs_slice(xt, k)[:, sl],
                                    start=(k == 0), stop=(k == KT - 1),
                                )
                            nc.scalar.activation(
                                ob[:, sl], ps[:, :P],
                                mybir.ActivationFunctionType.Identity,
                                bias=bias_sb[:, o:o + 1], scale=1.0,
                            )
                        if probe != "nodma":
                            nc.sync.dma_start(y_d.ap()[t, o], ob[:])
                        continue
                    ps = psum_tile()
                    for k in range(KT):
                        nc.tensor.matmul(
                            ps[:], w_slice(o, k), rhs_slice(xt, k),
                            start=(k == 0), stop=(k == KT - 1),
                        )
                    act_store(t, o, ps)

            def compute_chunk_stream(pieces):
                # t=0 single-shot: k-outer, o-inner over all 8 PSUM banks so
                # only the k-pieces that already arrived are needed.
                pss = [psum_tile() for o in range(OT)]
                for k in range(KT):
                    rhs = rhs_slice(pieces, k)
                    for o in range(OT):
                        nc.tensor.matmul(
                            pss[o][:], w_slice(o, k), rhs,
                            start=(k == 0), stop=(k == KT - 1),
                        )
                for o in range(OT):
                    act_store(0, o, pss[o])

            def body(first=None):
                if probe == "nodma":
                    tiles = list(resident)
                elif first is not None:
                    # single-shot: later chunks queued behind the weights on
                    # the same SP ring, so they arrive strictly after them
                    tiles = [first, load_xt(1), load_xt(2), load_xt(3)]
                else:
                    tiles = [load_xt(0)] + [None] * (TT - 1)
                for t in range(TT):
                    if probe != "nodma" and t + 1 < TT and tiles[t + 1] is None:
                        tiles[t + 1] = load_xt(t + 1)
                    if probe == "dmaonly":
                        continue
                    if t == 0 and first is not None:
                        compute_chunk_stream(tiles[0])
                    else:
                        compute_chunk(t, tiles[t])

            def body_pipe(xa, xb):
                # Runs inside For_i. Chunks 0/1 (xa/xb) were prefetched by
                # the previous trip; chunk t+2 (mod TT) is prefetched during
                # chunk t so the next trip's first chunks are resident when
                # the loop barrier drops. xpool slot rotation is consistent
                # across trips (4 allocs per body, bufs=4).
                tiles = [xa, xb, None, None]
                nxt = [None, None]
                for t in range(TT):
                    if t + 2 < TT:
                        tiles[t + 2] = load_xt(t + 2)
                    else:
                        nxt[t + 2 - TT] = load_xt(t + 2 - TT)
                    compute_chunk(t, tiles[t])
                return nxt

            if n_iter == 1:
                body(first_eighths)
            elif probe != "full":
                with tc.For_i(0, n_iter, 1,
                              hint_engines=tuple(mybir.ALL_ENGINES)):
                    body()
            else:
                assert n_iter % 2 == 0
                xa, xb = load_xt(0), load_xt(1)
                with tc.For_i(0, n_iter, 2,
                              hint_engines=tuple(mybir.ALL_ENGINES)):
                    xa, xb = body_pipe(xa, xb)
                    xa, xb = body_pipe(xa, xb)

    nc.compile()
    return nc


def make_in_maps(x, idx, weight, bias, A_pool, B_pool):
    """Host-side shard + LoRA fold + relayout. Returns per-core input dicts."""
    x = np.asarray(x, dtype=np.float32)
    idx = np.asarray(idx)
    weight = np.asarray(weight, dtype=np.float32)
    bias = np.asarray(bias, dtype=np.float32)
    A_pool = np.asarray(A_pool, dtype=np.float32)
    B_pool = np.asarray(B_pool, dtype=np.float32)

    bias_t = np.ascontiguousarray(bias.reshape(OT, P).T)  # [p, o_chunk]

    sel = idx.reshape(B).astype(np.int64)
    in_maps = []
    for c in range(NCORES):
        # fold the expert's rank-8 update into the base weight
        w_eff = weight + SCALE * (A_pool[sel[c]] @ B_pool[sel[c]]).T
        wt = np.ascontiguousarray(
            w_eff.reshape(OT, P, KT, P).transpose(2, 3, 0, 1).reshape(KT, P, OT * P)
        ).astype(BF16_NP)
        xt = np.ascontiguousarray(
            x[c].reshape(TT, TW, KT, P).transpose(0, 3, 2, 1).reshape(TT, P, KT * TW)
        ).astype(BF16_NP)
        in_maps.append({"xt": xt, "wt": wt, "bias": bias_t})
    return in_maps


def assemble_output(results):
    """Per-core y blocks [TT, OT, P, TW] -> full [B, N, DIM] fp32 output."""
    out = np.empty((B, N, DIM), dtype=np.float32)
    for c in range(NCORES):
        yb = np.asarray(results[c]["y"], dtype=np.float32)
        out[c] = yb.transpose(0, 3, 1, 2).reshape(N, DIM)
    return out


_PROGRAM_CACHE = {}


def _get_program(n_iter: int = 1):
    if n_iter not in _PROGRAM_CACHE:
        _PROGRAM_CACHE[n_iter] = build_program(n_iter)
    return _PROGRAM_CACHE[n_iter]


def kernel(x, idx, frozen_mask, weight, bias, A_pool, B_pool):
    # frozen_mask only affects gradients (stop_gradient); forward is identical.
    nc = _get_program(1)
    in_maps = make_in_maps(x, idx, weight, bias, A_pool, B_pool)
    res = run_bass_kernel_spmd(nc, in_maps, list(range(NCORES)))
    return assemble_output(res.results)
